# revision 10
# baseline (speedup 1.0000x reference)
"""PixelPrototypeDistanceLoss on 8 Trainium2 NeuronCores.

Math: for each pixel p with label lb_p != 19:
    logit_p = emb_pixel_p . segment_queue[lb_p]
    loss = mean((1 - logit_p)^2)  over valid pixels

With onehot[c,p] = (lb_p == c), padding classes (19..31) given zero
prototype vectors so ignored pixels select sim == 0:
    sum_p valid*(1-logit)^2 = count - 2*S1 + S2
count comes from the host (numpy popcount over labels); the kernel only
computes S1 = sum(onehot*sim) and S2 = sum((onehot*sim)^2).

Sharding: batch dim across the 8 cores (one image each).  Per core:
  sim tiles [19, cg] computed as QT.T @ X with X = emb[b] reshaped [256, N]
  (channels-first, no transpose).  Four pixel-stacks at PE col-group
  offsets 0/32/64/96 so the DVE sees [128, cg] blocks.  The onehot is
  built ON DEVICE: a 5-partition matmul E2.T @ [labels;ones] yields
  diff[32s+c, j] = lb - c in PSUM, gpsimd copies it to SBUF bf16, and the
  per-group scalar_tensor_tensor fuses (diff==0)*sim with the row-sum for
  S1.  ScalarE activation(Square) accumulates S2.  This removes the
  512 KB host onehot from the HBM stream (the binding resource).
Pipelining: emb cast to fp8-e4m3 on host; 10 tapered x tiles on the sync
HWDGE ring issued upfront; the two tiny meta DMAs (labels+E2, QT) ride
the scalar HWDGE ring in parallel so the x stream starts immediately.
Small leading tile -> early PE warmup; small trailing tiles -> short
drain after the last HBM byte.  PE ones-reduce packs the 20 partial
accumulators into one single-descriptor [1,20] output DMA.
Host: sums partials in f64 and forms (count - 2*S1 + S2)/count.
"""

import numpy as np
import ml_dtypes

import concourse.bacc as bacc
import concourse.mybir as mybir
from concourse.tile import TileContext
from concourse import bass_utils

# Problem dims (hardcoded per harness contract).
B, D, H, W, C = 8, 256, 128, 128, 19
NPX = H * W          # 16384 pixels per core (one batch image)
NCORES = 8
IGNORE = 19

CP = 32              # padded class count (PE tile_position granularity)

# x DMA tiles (pixel counts): small first tile -> early pipeline start,
# tapered tail -> short serial drain.  One compute group per tile;
# cg = n/4 <= 512 so every PSUM tile is a single bank.
XTILES = [1024, 2048, 2048, 2048, 2048, 2048, 2048, 1536, 1024, 512]
assert sum(XTILES) == NPX
NG = len(XTILES)
CGS = [n // 4 for n in XTILES]
OFFS = np.cumsum([0] + CGS).tolist()
LBB_COLS = NPX // 4  # total diff columns

EMB_DT = mybir.dt.float8e4
EMB_NP = ml_dtypes.float8_e4m3

LQ_LAB = 2 * LBB_COLS            # u8 cols of bf16 labels ([5, cg] per group)
LQ_COLS = LQ_LAB + 2 * 128       # + E2 [5, 128] bf16

_CACHE = {}


def _build():
    if "nc" in _CACHE:
        return _CACHE["nc"]
    nc = bacc.Bacc(
        "TRN2",
        target_bir_lowering=False,
        debug=False,
        enable_asserts=False,
    )
    # x packed on host as [128, 2*NPX]: group g's block at cols
    # [2*base_g, 2*base_g + 2*n), chunk k at block-local cols [k*n, (k+1)*n)
    x_t = nc.dram_tensor("x", [128, 2 * NPX], EMB_DT, kind="ExternalInput")
    # qt fp8 bytes: col 32k+c = QT[128k+p, c]; classes >= 19 are zero
    qt_t = nc.dram_tensor("qt", [128, 2 * CP], mybir.dt.uint8,
                          kind="ExternalInput")
    # labels(+ones row) bf16 per group, then E2 bf16 [5, 128]
    lbq_t = nc.dram_tensor("lbq", [5, LQ_COLS], mybir.dt.uint8,
                           kind="ExternalInput")
    out_t = nc.dram_tensor("out", [1, 2 * NG], mybir.dt.float32,
                           kind="ExternalOutput")

    x = x_t.ap()
    AO = mybir.AluOpType

    with TileContext(nc) as tc:
        with (
            tc.tile_pool(name="xp", bufs=1) as xpool,
            tc.tile_pool(name="mp", bufs=1) as mpool,
            tc.tile_pool(name="scr", bufs=4) as spool,
            tc.tile_pool(name="jnk", bufs=2) as jpool,
            tc.tile_pool(name="acc", bufs=1) as apool,
            tc.tile_pool(name="dps", bufs=3, space="PSUM") as dpool,
            tc.tile_pool(name="ps", bufs=4, space="PSUM") as pspool,
            tc.tile_pool(name="rps", bufs=1, space="PSUM") as rpool,
        ):
            # tiny meta DMAs on the scalar (ACT) HWDGE ring so the x
            # stream on the sync ring starts in parallel
            lbqt = mpool.tile([5, LQ_COLS], mybir.dt.uint8)
            nc.scalar.dma_start(lbqt[:, :], lbq_t.ap())
            qtt = mpool.tile([128, 2 * CP], mybir.dt.uint8)
            nc.scalar.dma_start(qtt[:, :], qt_t.ap())

            # x tiles resident; all DMAs issued upfront on the sync ring
            xt = {}
            base = 0
            for g, n in enumerate(XTILES):
                t = xpool.tile([128, 2 * n], EMB_DT, tag=f"xg{g}")
                nc.sync.dma_start(t[:, :], x[:, 2 * base:2 * base + 2 * n])
                xt[g] = t
                base += n

            qt_sb = qtt[:, :].bitcast(EMB_DT)
            e2_sb = lbqt[:, LQ_LAB:LQ_COLS].bitcast(mybir.dt.bfloat16)

            acc = apool.tile([128, 2 * NG], mybir.dt.float32)
            masks = mpool.tile([128, LBB_COLS], mybir.dt.bfloat16)

            def diff_group(g):
                cg = CGS[g]
                off = OFFS[g]
                dps = dpool.tile([128, cg], mybir.dt.float32, tag="dps")
                lab = lbqt[:, 2 * off:2 * (off + cg)].bitcast(
                    mybir.dt.bfloat16)
                nc.tensor.matmul(out=dps[:, :], lhsT=e2_sb, rhs=lab,
                                 start=True, stop=True, tile_position=(0, 0))
                # onehot mask = (lb - c == 0) into SBUF bf16 (DVE; runs 3
                # groups ahead of the STT that consumes it)
                nc.vector.tensor_scalar(masks[:, off:off + cg], dps[:, :],
                                        0.0, None, AO.is_equal)

            for g in range(3):
                diff_group(g)

            for g, n in enumerate(XTILES):
                cg = CGS[g]
                off = OFFS[g]
                ps = pspool.tile([128, cg], mybir.dt.float32, tag="ps")
                for s in range(4):
                    for k in range(2):
                        col = k * n + s * cg
                        nc.tensor.matmul(
                            out=ps[CP * s:CP * (s + 1), :],
                            lhsT=qt_sb[:, k * CP:(k + 1) * CP],
                            rhs=xt[g][:, col:col + cg],
                            start=(k == 0), stop=(k == 1),
                            tile_position=(0, CP * s))

                t1 = spool.tile([128, cg], mybir.dt.bfloat16, tag="t1")
                # t1 = mask * sim ; acc[:, g] = row-sum(t1)
                nc.vector.scalar_tensor_tensor(
                    out=t1[:, :], in0=masks[:, off:off + cg], scalar=1.0,
                    in1=ps[:, :], op0=AO.mult, op1=AO.mult,
                    accum_out=acc[:, g:g + 1])
                # acc[:, NG+g] = row-sum(t1^2) on the scalar engine
                jk = jpool.tile([128, cg], mybir.dt.bfloat16, tag="jk")
                nc.scalar.activation(
                    jk[:, :], t1[:, :], mybir.ActivationFunctionType.Square,
                    accum_out=acc[:, NG + g:NG + g + 1])
                if g + 3 < NG:
                    diff_group(g + 3)

            # partition-reduce accumulators on the (idle-at-tail) PE so the
            # output is one single-descriptor [1, 2*NG] DMA
            ones = nc.const_aps.aps[(mybir.dt.float32, 1.0)]
            rps = rpool.tile([128, 2 * NG], mybir.dt.float32, tag="rps")
            nc.tensor.matmul(out=rps[0:1, :], lhsT=ones, rhs=acc[:, :],
                             start=True, stop=True, tile_position=(0, 0))
            res = apool.tile([1, 2 * NG], mybir.dt.float32)
            nc.vector.tensor_copy(res[:, :], rps[0:1, :])
            nc.sync.dma_start(out_t.ap(), res[:, :])

    nc.compile()
    _CACHE["nc"] = nc
    return nc


def _prep_in_maps(emb, lb, segment_queue):
    emb = np.asarray(emb)
    lb = np.asarray(lb)
    q = np.asarray(segment_queue, dtype=np.float32)

    qt = np.zeros((D, CP), np.float32)
    qt[:, :C] = q.T
    # pack [2,128,CP] -> [128, 2*CP]: col 32k+c = QT[128k+p, c]
    qt = np.ascontiguousarray(
        qt.reshape(2, 128, CP).transpose(1, 0, 2).reshape(128, 2 * CP)
        .astype(EMB_NP)).view(np.uint8)

    # E2 [5, 128] bf16: row s in 0..3: 1.0 at cols 32s..32s+31; row 4: -c
    e2 = np.zeros((5, 128), np.float32)
    for s in range(4):
        e2[s, 32 * s:32 * (s + 1)] = 1.0
    e2[4, :] = -np.tile(np.arange(CP, dtype=np.float32), 4)
    e2 = e2.astype(ml_dtypes.bfloat16)

    in_maps = []
    for b in range(B):
        x8 = emb[b].reshape(2, 128, NPX).astype(EMB_NP)
        # pack per DMA tile: xb[p, 2*base + k*n + j] = x8[k, p, base + j]
        xb = np.empty((128, 2 * NPX), EMB_NP)
        base = 0
        for n in XTILES:
            blk = x8[:, :, base:base + n]            # [2, 128, n]
            xb[:, 2 * base:2 * base + 2 * n] = (
                blk.transpose(1, 0, 2).reshape(128, 2 * n))
            base += n
        lbf = lb[b].reshape(-1).astype(np.float32)
        # per group: [5, cg] bf16 = 4 stack label rows + ones row
        lbq = np.empty((5, LQ_COLS), np.uint8)
        base = 0
        for g, n in enumerate(XTILES):
            cg = CGS[g]
            blkl = np.empty((5, cg), np.float32)
            blkl[0:4, :] = lbf[base:base + n].reshape(4, cg)
            blkl[4, :] = 1.0
            lbq[:, 2 * OFFS[g]:2 * (OFFS[g] + cg)] = (
                blkl.astype(ml_dtypes.bfloat16).view(np.uint8))
            base += n
        lbq[:, LQ_LAB:] = e2.view(np.uint8)
        in_maps.append({
            "x": xb,
            "qt": qt,
            "lbq": np.ascontiguousarray(lbq),
        })
    return in_maps


def _reduce_outputs(results, count):
    s1 = 0.0
    s2 = 0.0
    for r in results:
        o = np.asarray(r["out"], dtype=np.float64)
        s1 += o[0, :NG].sum()
        s2 += o[0, NG:].sum()
    num = count - 2.0 * s1 + s2
    return np.float32(num / count)


def run_on_cores(inputs, **kwargs):
    """Run the bass kernel on cores 0-7; returns (loss, BassKernelResults).

    The device occasionally reports a transient NRT_EXEC_UNIT_UNRECOVERABLE
    on a run that succeeds on immediate retry; retry a couple of times.
    """
    nc = _build()
    in_maps = _prep_in_maps(**inputs)
    count = float(np.count_nonzero(np.asarray(inputs["lb"]) != IGNORE))
    last_err = None
    for _ in range(3):
        try:
            res = bass_utils.run_bass_kernel_spmd(
                nc, in_maps, core_ids=list(range(NCORES)), **kwargs)
            return _reduce_outputs(res.results, count), res
        except Exception as e:  # transient device wedge -> retry
            last_err = e
    raise last_err


def kernel(emb, lb, segment_queue):
    loss, _ = run_on_cores({"emb": emb, "lb": lb, "segment_queue": segment_queue})
    return loss


# revision 11
# speedup vs baseline: 1.0872x; 1.0872x over previous
"""PixelPrototypeDistanceLoss on 8 Trainium2 NeuronCores.

Math: for each pixel p with label lb_p != 19:
    logit_p = emb_pixel_p . segment_queue[lb_p]
    loss = mean((1 - logit_p)^2)  over valid pixels

Trick: with onehot[c,p] = (lb_p == c) for c in [0,19), ignored pixels match
nothing, so
    sum_p valid*(1-logit)^2 = count - 2*S1 + S2
with S1 = sum(sim*onehot), S2 = sum((sim*onehot)^2); count comes from the
host (numpy popcount over labels), so the kernel only produces S1/S2.

Sharding: batch dim across the 8 cores (one image each).  Per core:
  sim tiles [19, cg] computed as QT.T @ X with X = emb[b] reshaped [256, N]
  (already channels-first, no transpose needed).  Four pixel-stacks at PE
  col-group offsets 0/32/64/96 so the DVE sees [128, cg] blocks.  QT is
  zero-padded to 32 columns so every PSUM row is written.
  scalar_tensor_tensor fuses onehot*sim with the row-sum for S1; ScalarE
  activation(Square) accumulates S2 from the bf16 t1.
Pipelining: emb cast to fp8-e4m3 on host; 6 tapered x tiles issued
upfront on the sync HWDGE ring; the meta DMA (onehot+qt) rides the
scalar HWDGE ring in parallel so the x stream starts immediately and the
first compute group is never blocked on metadata.  Tapered trailing
tiles shorten the serial drain after the last HBM byte.  A PE
ones-reduce (reusing the framework's f32 ones const) packs the 12
partial accumulators into one single-descriptor [1,12] output DMA.
Host: sums the partials in f64 and forms (count - 2*S1 + S2)/count.
"""

import numpy as np
import ml_dtypes

import concourse.bacc as bacc
import concourse.mybir as mybir
from concourse.tile import TileContext
from concourse import bass_utils

# Problem dims (hardcoded per harness contract).
B, D, H, W, C = 8, 256, 128, 128, 19
NPX = H * W          # 16384 pixels per core (one batch image)
NCORES = 8
IGNORE = 19

CP = 32              # padded class count (PE tile_position granularity)

# x DMA tiles (pixel counts): tapered tail -> short serial drain after
# the last HBM byte.  One compute group per tile, cg = n/4.
XTILES = [2048, 4096, 4096, 4096, 1536, 512]
assert sum(XTILES) == NPX
NG = len(XTILES)
CGS = [n // 4 for n in XTILES]
OFFS = np.cumsum([0] + CGS).tolist()
LBB_COLS = NPX // 4  # total onehot columns

EMB_DT = mybir.dt.float8e4
EMB_NP = ml_dtypes.float8_e4m3

META_COLS = LBB_COLS + 2 * CP     # onehot u8 + qt fp8 bytes

_CACHE = {}


def _blocks(cg):
    """Split cg into matmul moving-dim blocks of <=512 (PSUM bank limit)."""
    out = []
    o = 0
    while o < cg:
        b = min(512, cg - o)
        out.append((o, b))
        o += b
    return out


def _build():
    if "nc" in _CACHE:
        return _CACHE["nc"]
    nc = bacc.Bacc(
        "TRN2",
        target_bir_lowering=False,
        debug=False,
        enable_asserts=False,
    )
    # x packed on host as [128, 2*NPX]: group g's block at cols
    # [2*base_g, 2*base_g + 2*n), chunk k at block-local cols [k*n, (k+1)*n)
    x_t = nc.dram_tensor("x", [128, 2 * NPX], EMB_DT, kind="ExternalInput")
    # meta: onehot u8 (group-blocked), then qt fp8 bytes
    # (col 32k+c = QT[128k+p, c]; classes >= 19 zero)
    meta_t = nc.dram_tensor("meta", [128, META_COLS], mybir.dt.uint8,
                            kind="ExternalInput")
    out_t = nc.dram_tensor("out", [1, 2 * NG], mybir.dt.float32,
                           kind="ExternalOutput")

    x = x_t.ap()
    AO = mybir.AluOpType

    with TileContext(nc) as tc:
        with (
            tc.tile_pool(name="xp", bufs=1) as xpool,
            tc.tile_pool(name="mp", bufs=1) as mpool,
            tc.tile_pool(name="scr", bufs=4) as spool,
            tc.tile_pool(name="jnk", bufs=2) as jpool,
            tc.tile_pool(name="acc", bufs=1) as apool,
            tc.tile_pool(name="ps", bufs=3, space="PSUM") as pspool,
            tc.tile_pool(name="rps", bufs=1, space="PSUM") as rpool,
        ):
            # meta on the scalar (ACT) HWDGE ring so the x stream on the
            # sync ring starts in parallel and qt/onehot land first
            metat = mpool.tile([128, META_COLS], mybir.dt.uint8)
            nc.scalar.dma_start(metat[:, :], meta_t.ap())

            # x tiles resident; all DMAs issued upfront on the sync ring
            xt = {}
            base = 0
            for g, n in enumerate(XTILES):
                t = xpool.tile([128, 2 * n], EMB_DT, tag=f"xg{g}")
                nc.sync.dma_start(t[:, :], x[:, 2 * base:2 * base + 2 * n])
                xt[g] = t
                base += n

            lbbt = metat[:, 0:LBB_COLS]
            qt_sb = metat[:, LBB_COLS:META_COLS].bitcast(EMB_DT)

            acc = apool.tile([128, 2 * NG], mybir.dt.float32)

            for g, n in enumerate(XTILES):
                cg = CGS[g]
                off = OFFS[g]
                ps = pspool.tile([128, cg], mybir.dt.float32, tag="ps")
                for s in range(4):
                    for (mo, fb) in _blocks(cg):
                        for k in range(2):
                            col = k * n + s * cg + mo
                            nc.tensor.matmul(
                                out=ps[CP * s:CP * (s + 1), mo:mo + fb],
                                lhsT=qt_sb[:, k * CP:(k + 1) * CP],
                                rhs=xt[g][:, col:col + fb],
                                start=(k == 0), stop=(k == 1),
                                tile_position=(0, CP * s))

                t1 = spool.tile([128, cg], mybir.dt.bfloat16, tag="t1")
                # t1 = onehot * sim ; acc[:, g] = row-sum(t1)
                nc.vector.scalar_tensor_tensor(
                    out=t1[:, :], in0=lbbt[:, off:off + cg], scalar=1.0,
                    in1=ps[:, :], op0=AO.mult, op1=AO.mult,
                    accum_out=acc[:, g:g + 1])
                # acc[:, NG+g] = row-sum(t1^2) on the scalar engine
                jk = jpool.tile([128, cg], mybir.dt.bfloat16, tag="jk")
                nc.scalar.activation(
                    jk[:, :], t1[:, :], mybir.ActivationFunctionType.Square,
                    accum_out=acc[:, NG + g:NG + g + 1])

            # partition-reduce accumulators on the (idle-at-tail) PE so the
            # output is one single-descriptor [1, 2*NG] DMA
            ones = nc.const_aps.aps[(mybir.dt.float32, 1.0)]
            rps = rpool.tile([128, 2 * NG], mybir.dt.float32, tag="rps")
            nc.tensor.matmul(out=rps[0:1, :], lhsT=ones, rhs=acc[:, :],
                             start=True, stop=True, tile_position=(0, 0))
            res = apool.tile([1, 2 * NG], mybir.dt.float32)
            nc.vector.tensor_copy(res[:, :], rps[0:1, :])
            nc.sync.dma_start(out_t.ap(), res[:, :])

    nc.compile()
    _CACHE["nc"] = nc
    return nc


def _prep_in_maps(emb, lb, segment_queue):
    emb = np.asarray(emb)
    lb = np.asarray(lb)
    q = np.asarray(segment_queue, dtype=np.float32)

    qt = np.zeros((D, CP), np.float32)
    qt[:, :C] = q.T
    # pack [2,128,CP] -> [128, 2*CP]: col 32k+c = QT[128k+p, c]
    qt = np.ascontiguousarray(
        qt.reshape(2, 128, CP).transpose(1, 0, 2).reshape(128, 2 * CP)
        .astype(EMB_NP)).view(np.uint8)

    cls_pat = np.where(np.arange(CP) < C, np.arange(CP), -1)  # [32]

    in_maps = []
    for b in range(B):
        x8 = emb[b].reshape(2, 128, NPX).astype(EMB_NP)
        # pack per DMA tile: xb[p, 2*base + k*n + j] = x8[k, p, base + j]
        xb = np.empty((128, 2 * NPX), EMB_NP)
        base = 0
        for n in XTILES:
            blk = x8[:, :, base:base + n]            # [2, 128, n]
            xb[:, 2 * base:2 * base + 2 * n] = (
                blk.transpose(1, 0, 2).reshape(128, 2 * n))
            base += n
        lbf = lb[b].reshape(-1).astype(np.float32)
        # onehot[32*s + c, off_g + j] = (lb[base_g + s*cg + j] == c)
        segs = []
        base = 0
        for g, n in enumerate(XTILES):
            cg = CGS[g]
            seg = lbf[base:base + n].reshape(4, 1, cg)
            segs.append((seg == cls_pat[None, :, None]).reshape(128, cg))
            base += n
        meta = np.empty((128, META_COLS), np.uint8)
        meta[:, :LBB_COLS] = np.concatenate(segs, axis=1).astype(np.uint8)
        meta[:, LBB_COLS:] = qt
        in_maps.append({
            "x": xb,
            "meta": np.ascontiguousarray(meta),
        })
    return in_maps


def _reduce_outputs(results, count):
    s1 = 0.0
    s2 = 0.0
    for r in results:
        o = np.asarray(r["out"], dtype=np.float64)
        s1 += o[0, :NG].sum()
        s2 += o[0, NG:].sum()
    num = count - 2.0 * s1 + s2
    return np.float32(num / count)


def run_on_cores(inputs, **kwargs):
    """Run the bass kernel on cores 0-7; returns (loss, BassKernelResults).

    The device occasionally reports a transient NRT_EXEC_UNIT_UNRECOVERABLE
    on a run that succeeds on immediate retry; retry a couple of times.
    """
    nc = _build()
    in_maps = _prep_in_maps(**inputs)
    count = float(np.count_nonzero(np.asarray(inputs["lb"]) != IGNORE))
    last_err = None
    for _ in range(3):
        try:
            res = bass_utils.run_bass_kernel_spmd(
                nc, in_maps, core_ids=list(range(NCORES)), **kwargs)
            return _reduce_outputs(res.results, count), res
        except Exception as e:  # transient device wedge -> retry
            last_err = e
    raise last_err


def kernel(emb, lb, segment_queue):
    loss, _ = run_on_cores({"emb": emb, "lb": lb, "segment_queue": segment_queue})
    return loss
